# revision 16
# baseline (speedup 1.0000x reference)
"""GQA attention (B=2,S=2048,D=1024,H=16,KH=4,HD=64) + RoPE + causal mask on 8 trn2 cores.

Sharding: core = (batch b, kv-group g).  Each core computes its 4 query heads'
attention against its single KV head and a partial output  O_g @ wo_g  [S, D];
the host sums the 4 partials per batch.

Per-core device pipeline (everything transposed so softmax-sum runs on the PE):
  - host passes x[b]^T so QKV projections contract D on partitions
  - head_dim of wq/wk is permuted on host to [evens, odds] so RoPE is two
    32-row blocks; scores are invariant to a consistent q/k head_dim permutation
  - RoPE: partition-swap copy (DMA) + 3 DVE ops with [cos,cos,..]/[-sin,+sin,..] tiles
  - scores computed transposed  S^T[k, q] = K^T(lhsT) x Q^T(rhs), fp32r matmuls
  - causal mask added INSIDE score PSUM via extra matmul  (-BIG*I) @ staircase01
  - softmax without max-subtraction (scores bounded); exp on ScalarE w/ scale=1/8
  - AV uses V augmented with a ones column: one accumulating matmul yields both
    O^T[64, q] and the softmax denominator row
  - normalization: DVE reciprocal of denom row, PE K=1 matmul broadcast across
    64 partitions, fused multiply on the PSUM->SBUF copy
  - wo projection consumes O^T chunks directly as lhsT; PSUM -> DRAM stores
"""

import os
import sys

import numpy as np

for _p in ("/opt/trn_rl_repo", "/root/.axon_site/_ro/trn_rl_repo"):
    if os.path.isdir(_p) and _p not in sys.path:
        sys.path.insert(0, _p)

from contextlib import ExitStack

import concourse.bass as bass
import concourse.tile as tile
from concourse import bacc as _bacc
from concourse import mybir
from concourse.bass_utils import run_bass_kernel_spmd

B, S, D = 2, 2048, 1024
H, KH, HD = 16, 4, 64
REP = H // KH          # 4 query heads per kv head
GH = REP               # heads per core
P = 128
QB = 512               # q block (matmul moving free dim)
NKT = S // P           # 16 key tiles
NQB = S // QB          # 4 q blocks
DCH = D // P           # 8 contraction chunks for D
BIG = 30000.0          # pre-scale additive mask magnitude

f32 = mybir.dt.float32
f32r = mybir.dt.float32r
bf16 = mybir.dt.bfloat16

LAST_EXEC_NS = None
LAST_PROFILE = None


def _classify_mask(mask):
    m = np.asarray(mask).reshape(S, S)
    if not m.any():
        return "none"
    tril = np.tril(np.ones((S, S), dtype=bool))
    if (m[tril] == 0.0).all() and (m[~tril] < -1e30).all():
        return "causal"
    return "general"


def _build_nc(mode):
    nc = bass.Bass()
    xT = nc.declare_dram_parameter("xT", [DCH, P, S], bf16, isOutput=False)
    wq = nc.declare_dram_parameter("wq", [DCH, P, GH * HD], bf16, isOutput=False)
    wk = nc.declare_dram_parameter("wk", [DCH, P, 2 * HD], bf16, isOutput=False)
    wv = nc.declare_dram_parameter("wv", [DCH, P, HD], bf16, isOutput=False)
    wo = nc.declare_dram_parameter("wo", [2, P, D], bf16, isOutput=False)
    cos = nc.declare_dram_parameter("cos", [P, S], f32, isOutput=False)
    sin = nc.declare_dram_parameter("sin", [P, S], f32, isOutput=False)
    stair = nc.declare_dram_parameter("stair", [P, 896], bf16, isOutput=False)
    negI = nc.declare_dram_parameter("negI", [P, P], bf16, isOutput=False)
    ones1 = nc.declare_dram_parameter("ones1", [1, HD], bf16, isOutput=False)
    if mode == "general":
        maskT = nc.declare_dram_parameter("maskT", [NKT, P, S], f32, isOutput=False)
    out = nc.declare_dram_parameter("out", [S, D], f32, isOutput=True)

    with tile.TileContext(nc) as tc, ExitStack() as ctx:
        const = ctx.enter_context(tc.tile_pool(name="const", bufs=1))
        big = ctx.enter_context(tc.tile_pool(name="big", bufs=1))
        work = ctx.enter_context(tc.tile_pool(name="work", bufs=2))
        ptp = ctx.enter_context(tc.tile_pool(name="ptp", bufs=3))
        psp = ctx.enter_context(tc.tile_pool(name="psp", bufs=2, space="PSUM"))
        stp = ctx.enter_context(tc.tile_pool(name="stp", bufs=3, space="PSUM"))
        avp = ctx.enter_context(tc.tile_pool(name="avp", bufs=2, space="PSUM"))
        rbp = ctx.enter_context(tc.tile_pool(name="rbp", bufs=1, space="PSUM"))

        # ---- constants / weights to SBUF ----
        # few DMAs per tile: consumers wait per DMA-queue semaphore, and walrus
        # rejects instructions with too many wait conditions
        xt_sb = big.tile([P, DCH, S], bf16, tag="xt")
        for sb in range(NQB):
            sl = slice(sb * QB, (sb + 1) * QB)
            nc.sync.dma_start(
                out=xt_sb[:, :, sl],
                in_=xT[:, :, sl].rearrange("c p s -> p c s"),
            )
        wq_sb = const.tile([P, DCH, GH * HD], bf16, tag="wq")
        wk_sb = const.tile([P, DCH, 2 * HD], bf16, tag="wk")
        wv_sb = const.tile([P, DCH, HD], bf16, tag="wv")
        wo_sb = const.tile([P, 2, D], bf16, tag="wo")
        nc.sync.dma_start(out=wq_sb, in_=wq[:, :, :].rearrange("c p f -> p c f"))
        nc.sync.dma_start(out=wk_sb, in_=wk[:, :, :].rearrange("c p f -> p c f"))
        nc.sync.dma_start(out=wv_sb, in_=wv[:, :, :].rearrange("c p f -> p c f"))
        nc.sync.dma_start(out=wo_sb, in_=wo[:, :, :].rearrange("c p f -> p c f"))
        cos_sb = const.tile([P, S], f32, tag="cos")
        sin_sb = const.tile([P, S], f32, tag="sin")
        nc.sync.dma_start(out=cos_sb, in_=cos[:, :])
        nc.sync.dma_start(out=sin_sb, in_=sin[:, :])
        stair_sb = const.tile([P, 896], bf16, tag="stair")
        negI_sb = const.tile([P, P], bf16, tag="negI")
        ones_sb = const.tile([1, HD], bf16, tag="ones1")
        nc.sync.dma_start(out=stair_sb, in_=stair[:, :])
        nc.sync.dma_start(out=negI_sb, in_=negI[:, :])
        nc.sync.dma_start(out=ones_sb, in_=ones1[:, :])

        QT = big.tile([P, 2, S], bf16, tag="QT")       # [2 chunks x 128, S]
        # kv head replicated on both 64-partition halves so lhsT base matches rhs
        KT = big.tile([P, S], bf16, tag="KT")
        V = big.tile([P, NKT, HD + 1], bf16, tag="V")  # keys on partitions + ones col
        OTC = big.tile([P, 2, S], bf16, tag="OTC")     # normalized O^T chunks
        nc.vector.memset(V[:, :, HD:HD + 1], 1.0)

        def rope(ps, out_ap, nrows, sl):
            # ps rows: per 64-group [evens(32), odds(32)]; swap 32-row halves.
            # DMA cannot read PSUM, so evacuate via DVE copy first.
            sb_ps = work.tile([P, QB], f32, tag="ropesb")
            nc.vector.tensor_copy(sb_ps[:nrows], ps[:nrows])
            tmp = work.tile([P, QB], f32, tag="ropetmp")
            for r0 in range(0, nrows, 64):
                nc.sync.dma_start(
                    out=tmp[r0:r0 + 32, :], in_=sb_ps[r0 + 32:r0 + 64, :])
                nc.sync.dma_start(
                    out=tmp[r0 + 32:r0 + 64, :], in_=sb_ps[r0:r0 + 32, :])
            ta = work.tile([P, QB], f32, tag="ropeta")
            tb = work.tile([P, QB], f32, tag="ropetb")
            nc.vector.tensor_mul(ta[:nrows], sb_ps[:nrows], cos_sb[:nrows, sl])
            nc.vector.tensor_mul(tb[:nrows], tmp[:nrows], sin_sb[:nrows, sl])
            nc.vector.tensor_add(out_ap, ta[:nrows], tb[:nrows])

        # ---- Q^T with RoPE ----
        for ch in range(2):
            for sb in range(NQB):
                sl = slice(sb * QB, (sb + 1) * QB)
                ps = psp.tile([P, QB], f32, tag="proj")
                for dc in range(DCH):
                    nc.tensor.matmul(
                        ps, lhsT=wq_sb[:, dc, ch * P:(ch + 1) * P],
                        rhs=xt_sb[:, dc, sl],
                        start=(dc == 0), stop=(dc == DCH - 1),
                    )
                rope(ps, QT[:, ch, sl], P, sl)

        # ---- K^T with RoPE ----
        for sb in range(NQB):
            sl = slice(sb * QB, (sb + 1) * QB)
            ps = psp.tile([P, QB], f32, tag="proj")
            for dc in range(DCH):
                nc.tensor.matmul(
                    ps, lhsT=wk_sb[:, dc, :], rhs=xt_sb[:, dc, sl],
                    start=(dc == 0), stop=(dc == DCH - 1),
                )
            rope(ps, KT[:, sl], P, sl)

        # ---- V (keys on partitions) ----
        for st_i in range(NKT):
            ps = psp.tile([P, HD], f32, tag="proj")
            for dc in range(DCH):
                nc.tensor.matmul(
                    ps, lhsT=xt_sb[:, dc, st_i * P:(st_i + 1) * P],
                    rhs=wv_sb[:, dc, :],
                    start=(dc == 0), stop=(dc == DCH - 1),
                )
            nc.vector.tensor_copy(V[:, st_i, 0:HD], ps)

        # ---- attention per (head, q block) ----
        for h in range(GH):
            ch, hr = h // 2, (h % 2) * 64
            for qb in range(NQB):
                q0 = qb * QB
                qsl = slice(q0, q0 + QB)
                nk = 4 * (qb + 1) if mode == "causal" else NKT
                av = avp.tile([HD + 1, QB], f32, tag="av")
                for kt in range(nk):
                    diag = mode == "causal" and kt >= nk - 4
                    st = stp.tile([P, QB], f32, tag="st")
                    nc.tensor.matmul(
                        st, lhsT=KT[hr:hr + 64, kt * P:(kt + 1) * P],
                        rhs=QT[hr:hr + 64, ch, qsl],
                        start=True, stop=not diag,
                    )
                    if diag:
                        off = 384 - 128 * (kt - (nk - 4))
                        nc.tensor.matmul(
                            st, lhsT=negI_sb, rhs=stair_sb[:, off:off + QB],
                            start=False, stop=True,
                        )
                    if mode == "general":
                        mt = work.tile([P, QB], f32, tag="maskt")
                        nc.sync.dma_start(out=mt, in_=maskT[kt, :, qsl])
                        nc.vector.tensor_add(st, st, mt)
                    pt = ptp.tile([P, QB], bf16, tag="pt")
                    nc.scalar.activation(
                        pt, st, mybir.ActivationFunctionType.Exp, scale=0.125
                    )
                    nc.tensor.matmul(
                        av, lhsT=V[:, kt, :], rhs=pt,
                        start=(kt == 0), stop=(kt == nk - 1),
                    )
                # normalize: r = 1/denom; broadcast to 64 partitions via K=1 matmul
                r1 = work.tile([1, QB], bf16, tag="r1")
                with nc.allow_low_precision(reason="f32r is 4-byte fp32 bits"):
                    nc.vector.reciprocal(r1, av[HD:HD + 1, :])
                rb = rbp.tile([64, QB], f32, tag="rb")
                nc.tensor.matmul(rb, lhsT=ones_sb, rhs=r1, start=True, stop=True)
                rbs = work.tile([64, QB], f32, tag="rbs")
                nc.vector.tensor_copy(rbs, rb)
                ot = work.tile([64, QB], bf16, tag="ot")
                nc.vector.tensor_mul(ot, av[0:HD, :], rbs)
                # gpsimd SWDGE is pinned to one queue -> single wait condition
                # for the wo matmuls that consume OTC
                nc.gpsimd.dma_start(out=OTC[hr:hr + 64, ch, qsl], in_=ot)

        # ---- output projection: out[q,:] = sum_c OTC[:,c,q].T @ wo[c] ----
        for qt in range(NKT):
            for dh in range(2):
                ps = psp.tile([P, 512], f32, tag="proj")
                for c in range(2):
                    nc.tensor.matmul(
                        ps, lhsT=OTC[:, c, qt * P:(qt + 1) * P],
                        rhs=wo_sb[:, c, dh * 512:(dh + 1) * 512],
                        start=(c == 0), stop=(c == 1),
                    )
                osb = work.tile([P, 512], f32, tag="osb")
                nc.vector.tensor_copy(osb, ps)
                nc.sync.dma_start(
                    out=out[qt * P:(qt + 1) * P, dh * 512:(dh + 1) * 512], in_=osb
                )
    # split multi-wait conditions: TRN2 instructions hold at most one sync
    # wait (EventSemaphore holds two); walrus refuses to split them itself
    import bass_rust
    bass_rust.move_matmul_waits_to_ldweights(nc.m)
    bass_rust.generate_event_semaphores(nc)
    return nc


_NC_CACHE = {}


def kernel(_trace=False, **inputs):
    global LAST_EXEC_NS, LAST_PROFILE
    x = np.ascontiguousarray(np.asarray(inputs["x"], dtype=np.float32))
    wq = np.asarray(inputs["wq"], dtype=np.float32)
    wk = np.asarray(inputs["wk"], dtype=np.float32)
    wv = np.asarray(inputs["wv"], dtype=np.float32)
    wo = np.asarray(inputs["wo"], dtype=np.float32)
    fc = np.asarray(inputs["freqs_cos"], dtype=np.float32)
    fs = np.asarray(inputs["freqs_sin"], dtype=np.float32)
    mask = np.asarray(inputs["mask"], dtype=np.float32)

    mode = _classify_mask(mask)
    if mode not in _NC_CACHE:
        _NC_CACHE[mode] = _build_nc(mode)
    nc = _NC_CACHE[mode]
    in_maps = _make_in_maps(x, wq, wk, wv, wo, fc, fs, mask, mode)

    try:
        res = run_bass_kernel_spmd(
            nc, in_maps, core_ids=list(range(8)), trace=_trace)
    except (ModuleNotFoundError, ImportError):
        res = run_bass_kernel_spmd(
            nc, in_maps, core_ids=list(range(8)), trace=False)
    LAST_EXEC_NS = res.exec_time_ns
    LAST_PROFILE = res.profile_json
    full = np.zeros((B, S, D), dtype=np.float32)
    for b in range(B):
        for g in range(KH):
            full[b] += res.results[b * KH + g]["out"]
    return full


def _make_in_maps(x, wq, wk, wv, wo, fc, fs, mask, mode):
    # head_dim permutation: evens then odds (consistent on q & k -> scores invariant)
    perm = np.concatenate([np.arange(0, HD, 2), np.arange(1, HD, 2)])
    wq_p = wq.reshape(D, H, HD)[:, :, perm].reshape(D, H * HD)
    wk_p = wk.reshape(D, KH, HD)[:, :, perm].reshape(D, KH * HD)

    cosT = fc.T.astype(np.float32)                      # [32, S]
    sinT = fs.T.astype(np.float32)
    cos_rep = np.ascontiguousarray(np.tile(cosT, (4, 1)))          # [128, S]
    sin_signed = np.ascontiguousarray(
        np.concatenate([-sinT, sinT, -sinT, sinT], axis=0))        # [128, S]

    cc = np.arange(P)[:, None]
    mm = np.arange(896)[None, :]
    stair = (cc > (mm - 384)).astype(np.float32)
    negI = (-BIG * np.eye(P)).astype(np.float32)
    ones1 = np.ones((1, HD), dtype=np.float32)

    import ml_dtypes
    b16 = ml_dtypes.bfloat16

    in_maps = []
    for b in range(B):
        xTb = np.ascontiguousarray(x[b].T).astype(b16).reshape(DCH, P, S)
        for g in range(KH):
            wk_g = wk_p[:, g * HD:(g + 1) * HD]
            wk_dup = np.concatenate([wk_g, wk_g], axis=1)       # [D, 128]
            m = {
                "xT": xTb,
                "wq": np.ascontiguousarray(
                    wq_p[:, g * GH * HD:(g + 1) * GH * HD]
                ).astype(b16).reshape(DCH, P, GH * HD),
                "wk": np.ascontiguousarray(wk_dup).astype(b16).reshape(DCH, P, 2 * HD),
                "wv": np.ascontiguousarray(
                    wv[:, g * HD:(g + 1) * HD]).astype(b16).reshape(DCH, P, HD),
                "wo": np.ascontiguousarray(
                    wo[g * GH * HD:(g + 1) * GH * HD]).astype(b16).reshape(2, P, D),
                "cos": cos_rep,
                "sin": sin_signed,
                "stair": stair.astype(b16),
                "negI": negI.astype(b16),
                "ones1": ones1.astype(b16),
            }
            if mode == "general":
                m["maskT"] = np.ascontiguousarray(
                    mask.reshape(S, S).T).reshape(NKT, P, S)
            in_maps.append(m)
    return in_maps
